# revision 14
# baseline (speedup 1.0000x reference)
"""Trainium2 Bass kernel for the JointLoss problem (contrastive NT-Xent + 2 MSE terms).

kernel(representation, xrecon, xorig) -> (loss, closs, recon_loss, zrecon_loss)

Strategy (8 NeuronCores, SPMD — one NEFF, per-core variation only via inputs):
  - Host normalizes the representations (the sharding hint's "all-gather of the
    normalized representations" — normalization happens before the gather in the
    data-parallel formulation), scales by S=16 so fp8 e4m3 stays in its normal
    range, and ships z^T slabs per core.  sim/tau is recovered by folding
    1/(S^2 tau) into the exp's constant scale.
  - Each core computes a (512, 2560) slab of q = (S z_i)·(S z_j) against column
    chunks [partner, own, +1, +2, +3] using fp8 DoubleRow matmuls (K=256 per
    instruction).  exp runs on Act with per-call row-sum accumulators; chunks
    +1..+3 write fp8 exp tiles whose column sums (one DoubleRow ones-matmul
    pair per chunk) supply the transposed contributions to other cores' rows.
  - Positives come from the diagonal of the partner block, extracted from PSUM
    with an identity mask multiply + free-axis reduce on DVE.  (The native
    tensor_tensor_reduce ISA op crashes the exec unit on this platform.)
  - MSE terms: bf16 subtract + square on DVE; the full-tile sums run as
    single cross-partition XYZWC reduces on the otherwise-idle Pool engine.
  - Host combine: sum the 8 cores' partial row sums + pushed column sums,
    subtract exp(1/tau) for the self column, log, and the two MSE scalars.
"""

import math

import ml_dtypes
import numpy as np

TAU = 0.5
EPS = 1e-8
N = 2048
TWO_N = 4096
D = 512
NCORES = 8
CH = 512
S = 16.0  # fp8 pre-scale for normalized vectors
QS = 1.0 / (S * S * TAU)  # exp input scale

_CACHE = {}


def _build_nc():
    import concourse.bacc as bacc
    import concourse.mybir as mybir
    import concourse.tile as tile
    from concourse.masks import make_identity

    F32 = mybir.dt.float32
    BF16 = mybir.dt.bfloat16
    FP8 = mybir.dt.float8e4
    OP = mybir.AluOpType
    AF = mybir.ActivationFunctionType
    AX = mybir.AxisListType
    DR = mybir.MatmulPerfMode.DoubleRow

    nc = bacc.Bacc("TRN2", target_bir_lowering=False, debug=False)
    # zt[ch][p][d*512+c] = (S*z)^T[d*128+p, 512*ch + c] (permuted cols, 5 chunks)
    zt = nc.dram_tensor("zt", [5, 128, 2048], FP8, kind="ExternalInput")
    xr = nc.dram_tensor("xr", [128, 4096], BF16, kind="ExternalInput")
    xo = nc.dram_tensor("xo", [128, 4096], BF16, kind="ExternalInput")
    zi = nc.dram_tensor("zi", [128, 1024], BF16, kind="ExternalInput")
    zj = nc.dram_tensor("zj", [128, 1024], BF16, kind="ExternalInput")
    out1 = nc.dram_tensor("out1", [128, 20], F32, kind="ExternalOutput")
    ocol = nc.dram_tensor("ocol", [1, 1536], F32, kind="ExternalOutput")

    with tile.TileContext(nc) as tc:
        with (
            tc.tile_pool(name="singles", bufs=1) as singles,
            tc.tile_pool(name="ebap", bufs=2) as ebap,
            tc.tile_pool(name="msep", bufs=2) as msep,
            tc.tile_pool(name="mpsum", bufs=2, space="PSUM") as mpsum,
            tc.tile_pool(name="cpsum", bufs=1, space="PSUM") as cpsum,
            tc.tile_pool(name="capsum", bufs=1, space="PSUM") as capsum,
        ):
            identf = singles.tile([128, 128], F32, tag="identf")
            make_identity(nc, identf)
            # dual-fp8 ldweights requires the k-pair stride in the weights AP
            # to be 16-byte aligned (walrus s3_lw_dual_fp8_restrictions), so
            # the ones live in a [128, 2, 16] tile sliced to [:, :, 0:2]
            ones8 = singles.tile([128, 2, 16], FP8, tag="ones8")
            nc.vector.memset(ones8, 1.0)
            # acc columns: 0-11 eacc[rr*3+blk], 12-15 pos[rr], 16-17 dx halves,
            # 18 dz, 19 unused
            acc = singles.tile([128, 20], F32, tag="acc")
            eb = singles.tile([128, 4, 1536], FP8, tag="eb")

            # input tiles + DMA (zt chunks 0/1 split at the d=2 boundary so the
            # dd0 matmuls can start as soon as the first half lands)
            zt_t = []
            for ch in range(5):
                t = singles.tile([128, 4, 512], FP8, tag=f"zt_{ch}")
                if ch < 2:
                    nc.sync.dma_start(t[:, 0:2, :], zt[ch][:, 0:1024])
                    nc.sync.dma_start(t[:, 2:4, :], zt[ch][:, 1024:2048])
                else:
                    nc.sync.dma_start(t, zt[ch])
                zt_t.append(t)

            zit = singles.tile([128, 1024], BF16, tag="zit")
            nc.gpsimd.dma_start(zit, zi[:, :])
            zjt = singles.tile([128, 1024], BF16, tag="zjt")
            nc.gpsimd.dma_start(zjt, zj[:, :])
            xrt = singles.tile([128, 4096], BF16, tag="xrt")
            xot = singles.tile([128, 4096], BF16, tag="xot")
            nc.gpsimd.dma_start(xrt[:, 0:2048], xr[:, 0:2048])
            nc.gpsimd.dma_start(xot[:, 0:2048], xo[:, 0:2048])
            nc.gpsimd.dma_start(xrt[:, 2048:4096], xr[:, 2048:4096])
            nc.gpsimd.dma_start(xot[:, 2048:4096], xo[:, 2048:4096])

            for rr in range(4):
                psA = mpsum.tile([128, 1024], F32, tag="ps")
                psB = mpsum.tile([128, 1024], F32, tag="ps")
                psC = cpsum.tile([128, 512], F32, tag="psC")
                for dd in range(2):
                    w = zt_t[1][:, 2 * dd : 2 * dd + 2, 128 * rr : 128 * (rr + 1)]
                    for ch in range(5):
                        if ch < 2:
                            dst = psA[:, CH * ch : CH * (ch + 1)]
                        elif ch < 4:
                            dst = psB[:, CH * (ch - 2) : CH * (ch - 1)]
                        else:
                            dst = psC
                        nc.tensor.matmul(
                            dst,
                            w,
                            zt_t[ch][:, 2 * dd : 2 * dd + 2, :],
                            start=(dd == 0),
                            stop=(dd == 1),
                            perf_mode=DR,
                        )
                # positives: diagonal of the partner block (raw q, pre-exp)
                ext = msep.tile([128, 128], F32, tag="ext")
                nc.vector.tensor_tensor(
                    ext, psA[:, 128 * rr : 128 * (rr + 1)], identf, OP.mult
                )
                nc.vector.reduce_sum(acc[:, 12 + rr : 13 + rr], ext, axis=AX.X)
                ebA = ebap.tile([128, 1024], FP8, tag="ebA")
                nc.scalar.activation(
                    ebA, psA, AF.Exp, scale=QS, accum_out=acc[:, 3 * rr : 3 * rr + 1]
                )
                nc.scalar.activation(
                    eb[:, rr, 0:1024],
                    psB,
                    AF.Exp,
                    scale=QS,
                    accum_out=acc[:, 3 * rr + 1 : 3 * rr + 2],
                )
                nc.scalar.activation(
                    eb[:, rr, 1024:1536],
                    psC,
                    AF.Exp,
                    scale=QS,
                    accum_out=acc[:, 3 * rr + 2 : 3 * rr + 3],
                )

            # column sums of chunks +1/+2/+3 exp tiles (DoubleRow over rr pairs),
            # staged PSUM -> SBUF per segment (PSUM is not DMA-able)
            cap = capsum.tile([2, 1536], F32, tag="cap")
            ocolt = singles.tile([1, 1536], F32, tag="ocolt")
            for ch in range(3):
                for j in range(2):
                    nc.tensor.matmul(
                        cap[0:2, CH * ch : CH * (ch + 1)],
                        ones8[:, :, 0:2],
                        eb[:, 2 * j : 2 * j + 2, CH * ch : CH * (ch + 1)],
                        start=(j == 0),
                        stop=(j == 1),
                        perf_mode=DR,
                    )
                nc.vector.tensor_copy(
                    ocolt[0:1, CH * ch : CH * (ch + 1)],
                    cap[0:1, CH * ch : CH * (ch + 1)],
                )

            # MSE partials: bf16 subtract + square on DVE, full-tile sum on Pool
            dxs = singles.tile([128, 4096], BF16, tag="dxs")
            dzs = singles.tile([128, 1024], BF16, tag="dzs")

            def mse(a, b, sq, tag):
                w = a.shape[-1]
                d = msep.tile([128, 2048], BF16, tag="d")
                nc.vector.tensor_tensor(d[:, 0:w], a, b, OP.subtract)
                nc.vector.tensor_tensor(sq, d[:, 0:w], d[:, 0:w], OP.mult)

            mse(zit, zjt, dzs, "z")
            mse(xrt[:, 0:2048], xot[:, 0:2048], dxs[:, 0:2048], "x0")
            mse(xrt[:, 2048:4096], xot[:, 2048:4096], dxs[:, 2048:4096], "x1")
            nc.gpsimd.tensor_reduce(acc[0:1, 16:17], dxs, axis=AX.XYZWC, op=OP.add)
            nc.gpsimd.tensor_reduce(acc[0:1, 17:18], dzs, axis=AX.XYZWC, op=OP.add)

            nc.sync.dma_start(out1[:, :], acc)
            nc.sync.dma_start(ocol[:, :], ocolt)

    nc.compile()
    return nc


def _get_nc():
    if "nc" not in _CACHE:
        _CACHE["nc"] = _build_nc()
    return _CACHE["nc"]


def make_in_maps(representation, xrecon, xorig):
    rep = np.ascontiguousarray(np.asarray(representation, dtype=np.float32))
    nrm = np.maximum(np.linalg.norm(rep, axis=1, keepdims=True), EPS)
    u = (rep / nrm) * S
    uq = u.astype(ml_dtypes.float8_e4m3)
    UT = np.ascontiguousarray(uq.T)  # (512, 4096) fp8
    xrec = np.asarray(xrecon, dtype=np.float32).astype(ml_dtypes.bfloat16)
    xorg = np.asarray(xorig, dtype=np.float32).astype(ml_dtypes.bfloat16)
    repb = rep.astype(ml_dtypes.bfloat16)
    in_maps = []
    for c in range(NCORES):
        partner = (c + 4) % 8
        order = [partner, c, (c + 1) % 8, (c + 2) % 8, (c + 3) % 8]
        ut_c = np.concatenate([UT[:, CH * p : CH * (p + 1)] for p in order], axis=1)
        # [d, p, ch, col] -> [ch, p, d, col]  (ch = 512-col chunk index)
        zt_c = np.ascontiguousarray(
            ut_c.reshape(4, 128, 5, 512).transpose(2, 1, 0, 3).reshape(5, 128, 2048)
        )
        in_maps.append(
            {
                "zt": zt_c,
                "xr": np.ascontiguousarray(
                    xrec[CH * c : CH * (c + 1)]
                    .reshape(4, 128, 1024).transpose(1, 0, 2).reshape(128, 4096)
                ),
                "xo": np.ascontiguousarray(
                    xorg[CH * c : CH * (c + 1)]
                    .reshape(4, 128, 1024).transpose(1, 0, 2).reshape(128, 4096)
                ),
                "zi": np.ascontiguousarray(
                    repb[256 * c : 256 * (c + 1)]
                    .reshape(2, 128, D).transpose(1, 0, 2).reshape(128, 1024)
                ),
                "zj": np.ascontiguousarray(
                    repb[2048 + 256 * c : 2048 + 256 * (c + 1)]
                    .reshape(2, 128, D).transpose(1, 0, 2).reshape(128, 1024)
                ),
            }
        )
    return in_maps


def combine_outputs(results):
    """results: list of 8 dicts with out1 [128,20], ocol [1,1536]."""
    E2 = math.exp(1.0 / TAU)
    denom = np.zeros(TWO_N, dtype=np.float64)
    pos = np.zeros(TWO_N, dtype=np.float64)
    dxs = 0.0
    dzs = 0.0
    for c in range(NCORES):
        a = np.asarray(results[c]["out1"], dtype=np.float64)  # [128, 20]
        # partition p, row group rr -> global row 512c + 128rr + p
        rsum = a[:, 0:12].reshape(128, 4, 3).sum(axis=2)  # [p, rr]
        denom[CH * c : CH * (c + 1)] += rsum.T.reshape(-1)
        pos[CH * c : CH * (c + 1)] = a[:, 12:16].T.reshape(-1)
        oc = np.asarray(results[c]["ocol"], dtype=np.float64).reshape(3, CH)
        for k in range(3):
            m = (c + 1 + k) % NCORES
            denom[CH * m : CH * (m + 1)] += oc[k]
        dxs += a[0, 16]
        dzs += a[0, 17]
    denom -= E2
    closs = (np.log(denom) - pos * QS).sum() / TWO_N
    recon = dxs / TWO_N
    zrec = dzs / N
    loss = recon + closs + zrec
    f = np.float32
    return (f(loss), f(closs), f(recon), f(zrec))


def kernel(representation, xrecon, xorig):
    from concourse.bass_utils import run_bass_kernel_spmd

    nc = _get_nc()
    in_maps = make_in_maps(representation, xrecon, xorig)
    res = run_bass_kernel_spmd(nc, in_maps, core_ids=list(range(NCORES)))
    return combine_outputs(res.results)
